# revision 1
# baseline (speedup 1.0000x reference)
"""Dynamic Influence Model kernel: builder + host glue.

Device strategy (per core, 8 cores data-parallel over batch B=64):
  - Host builds a sequence-major table: bt[a] = concat_t emb[t, align[a, t]]
    (bf16, [50002, T*128]) with zero rows at 0 and 32768 so that two int16
    dma_gather(transpose=True) calls + one DVE add produce exact gathers of
    50000-row-indexed sequences directly in x^T layout [d, t, m].
  - L2-normalize over the 64-neighbor groups (DVE square/segment-reduce +
    Newton rsqrt on DVE; no ACT table switches).
  - BiLSTM over T steps: PE matmuls (bf16, psum f32), gate activations on
    ACT with fused per-partition bias, c/h updates on DVE.
  - sum_nb relu(h_final) per (relation, direction) -> [128, 8] f32 out.
Host: final small FC chain in float64 (exactly equivalent algebra: the
neighbor-sum is hoisted through the linear layers).
"""
import numpy as np
import ml_dtypes
from dataclasses import dataclass

import concourse.bass as bass
from concourse import mybir, bacc
from concourse.tile import TileContext, add_dep_helper

F32 = mybir.dt.float32
BF16 = mybir.dt.bfloat16
I16 = mybir.dt.int16
AF = mybir.ActivationFunctionType
OP = mybir.AluOpType


@dataclass
class Cfg:
    R: int = 3
    T: int = 16
    D: int = 128
    M: int = 512          # sequences per core (= 8 batch * 64 nb)
    NBG: int = 8          # neighbor groups per core (M / 64)
    NROWS_RAW: int = 50000
    c_f32: bool = False   # keep LSTM cell state in f32 (slower, more exact)
    gates_f32: bool = False

    @property
    def NROWS(self):      # + 2 zero rows
        return self.NROWS_RAW + 2

    @property
    def ELEM(self):       # bf16 values per table row
        return self.T * self.D

    @property
    def EHALF(self):
        return self.ELEM // 2

    @property
    def TH(self):         # timesteps per gather call
        return self.T // 2


def build_nc(cfg: Cfg):
    R, T, D, M = cfg.R, cfg.T, cfg.D, cfg.M
    H = D
    CDT = F32 if cfg.c_f32 else BF16
    GDT = F32 if cfg.gates_f32 else BF16
    SPLIT = 32768 if cfg.NROWS_RAW > 32768 else (cfg.NROWS_RAW // 2 + 1)

    nc = bacc.Bacc("TRN2", target_bir_lowering=False, num_devices=8,
                   dynamic_dma_scratch_size=32768)
    table = nc.dram_tensor("table", [cfg.NROWS, cfg.ELEM], BF16, kind="ExternalInput")
    idxs = nc.dram_tensor("idxs", [128, R, 2, M // 16], I16, kind="ExternalInput")
    wih = nc.dram_tensor("wih", [128, R, 2, 4 * H], BF16, kind="ExternalInput")
    whh = nc.dram_tensor("whh", [128, R, 2, 4 * H], BF16, kind="ExternalInput")
    bias = nc.dram_tensor("bias", [128, R, 2, 4], F32, kind="ExternalInput")
    sout = nc.dram_tensor("sout", [R, 2, 128, cfg.NBG], F32, kind="ExternalOutput")

    with TileContext(nc) as tc:
        with tc.tile_pool(name="const", bufs=1) as cp, \
             tc.tile_pool(name="xp", bufs=1) as xp, \
             tc.tile_pool(name="gp", bufs=4) as gp, \
             tc.tile_pool(name="sqp", bufs=1) as sqp, \
             tc.tile_pool(name="nt", bufs=3) as ntp, \
             tc.tile_pool(name="st", bufs=4) as st, \
             tc.tile_pool(name="gt", bufs=4) as gtp, \
             tc.tile_pool(name="ps", bufs=1, space="PSUM") as psp:

            warm_i = cp.tile([128, 8], I16)
            nc.gpsimd.memset(warm_i[:], 0)
            warm_o = cp.tile([128, 1, 128], BF16)
            nc.gpsimd.dma_gather(
                out_ap=warm_o[:], in_ap=table[0:256, 0:128], idxs_ap=warm_i[:, :],
                num_idxs=128, num_idxs_reg=128, elem_size=128, elem_step=cfg.ELEM,
                transpose=True)
            it = cp.tile([128, R, 2, M // 16], I16)
            nc.sync.dma_start(out=it[:], in_=idxs[:])
            wih_t = cp.tile([128, R, 2, 4 * H], BF16)
            nc.sync.dma_start(out=wih_t[:], in_=wih[:])
            whh_t = cp.tile([128, R, 2, 4 * H], BF16)
            nc.sync.dma_start(out=whh_t[:], in_=whh[:])
            bias_t = cp.tile([128, R, 2, 4], F32)
            nc.sync.dma_start(out=bias_t[:], in_=bias[:])

            def gather_half(r, e, xr):
                g1 = gp.tile([128, cfg.TH, M], BF16, tag="g", name="g1")
                g2 = gp.tile([128, cfg.TH, M], BF16, tag="g", name="g2")
                nc.gpsimd.dma_gather(
                    out_ap=g1[:], in_ap=table[0:SPLIT, e * cfg.EHALF:(e + 1) * cfg.EHALF],
                    idxs_ap=it[:, r, 0, :], num_idxs=M, num_idxs_reg=M,
                    elem_size=cfg.EHALF, elem_step=cfg.ELEM, transpose=True,
                )
                nc.gpsimd.dma_gather(
                    out_ap=g2[:], in_ap=table[SPLIT:cfg.NROWS, e * cfg.EHALF:(e + 1) * cfg.EHALF],
                    idxs_ap=it[:, r, 1, :], num_idxs=M, num_idxs_reg=M,
                    elem_size=cfg.EHALF, elem_step=cfg.ELEM, transpose=True,
                )
                nch = 4 if cfg.TH % 4 == 0 else 2
                th2 = cfg.TH // nch
                insts = []
                for k in range(nch):
                    insts.append(nc.vector.tensor_tensor(
                        out=xr[:, e * cfg.TH + k * th2:e * cfg.TH + (k + 1) * th2, :],
                        in0=g1[:, k * th2:(k + 1) * th2, :],
                        in1=g2[:, k * th2:(k + 1) * th2, :], op=OP.add))
                return insts

            def norm_half(r, e, xr, ss):
                sq = sqp.tile([128, cfg.TH, M], BF16, tag="sq", name="sq")
                xe = xr[:, e * cfg.TH:(e + 1) * cfg.TH, :]
                nch = 4 if cfg.TH % 4 == 0 else 2
                th2 = cfg.TH // nch
                for k in range(nch):
                    nc.vector.tensor_tensor(out=sq[:, k * th2:(k + 1) * th2, :],
                                            in0=xe[:, k * th2:(k + 1) * th2, :],
                                            in1=xe[:, k * th2:(k + 1) * th2, :], op=OP.mult)
                for k in range(2):
                    th3 = cfg.TH // 2
                    nc.vector.tensor_reduce(
                        out=ss[:, (e * cfg.TH + k * th3) * cfg.NBG:(e * cfg.TH + (k + 1) * th3) * cfg.NBG],
                        in_=sq[:, k * th3:(k + 1) * th3, :].rearrange("p t (b n) -> p (t b) n", n=64),
                        op=OP.add, axis=mybir.AxisListType.X)

            def newton(ss_ap, y_ap):
                # y = min(rsqrt(ss), 1e12), DVE-only (bit trick + 2 Newton iters)
                n = ss_ap.ap[-1][1]
                ssi = ss_ap.bitcast(mybir.dt.int32)
                yi = y_ap.bitcast(mybir.dt.int32)
                nc.vector.tensor_scalar(out=yi, in0=ssi, scalar1=1, scalar2=None,
                                        op0=OP.logical_shift_right)
                nc.vector.tensor_scalar(out=yi, in0=yi, scalar1=-1, scalar2=0x5f3759df,
                                        op0=OP.mult, op1=OP.add)
                t1 = ntp.tile([128, n], F32, tag="nt1", name="nt1")
                for _ in range(1):
                    nc.vector.tensor_tensor(out=t1[:], in0=y_ap, in1=y_ap, op=OP.mult)
                    nc.vector.tensor_tensor(out=t1[:], in0=t1[:], in1=ss_ap, op=OP.mult)
                    nc.vector.tensor_scalar(out=t1[:], in0=t1[:], scalar1=-0.5, scalar2=1.5,
                                            op0=OP.mult, op1=OP.add)
                    nc.vector.tensor_tensor(out=y_ap, in0=y_ap, in1=t1[:], op=OP.mult)
                # no clamp needed: ss=0 yields a large finite y and x=0 -> x*y=0

            def scale_half(r, e, xr, y):
                nch = 4 if cfg.TH % 4 == 0 else 2
                th2 = cfg.TH // nch
                last = None
                for k in range(nch):
                    sv = bass.AP(y.tensor, y.offset + (e * cfg.TH + k * th2) * cfg.NBG,
                                 [y.ap[0], [cfg.NBG, th2], [1, cfg.NBG], [0, 64]])
                    xe = xr[:, e * cfg.TH + k * th2:e * cfg.TH + (k + 1) * th2, :]
                    last = nc.vector.tensor_tensor(
                        out=xe.rearrange("p t (b n) -> p t b n", n=64),
                        in0=xe.rearrange("p t (b n) -> p t b n", n=64),
                        in1=sv, op=OP.mult)
                return last

            def gather_sub(r, tlo, thi, xr):
                nt = thi - tlo
                g1 = gp.tile([128, nt, M], BF16, tag="gs", name="g1s")
                g2 = gp.tile([128, nt, M], BF16, tag="gs", name="g2s")
                nc.gpsimd.dma_gather(
                    out_ap=g1[:], in_ap=table[0:SPLIT, tlo * D:thi * D],
                    idxs_ap=it[:, r, 0, :], num_idxs=M, num_idxs_reg=M,
                    elem_size=nt * D, elem_step=cfg.ELEM, transpose=True,
                )
                nc.gpsimd.dma_gather(
                    out_ap=g2[:], in_ap=table[SPLIT:cfg.NROWS, tlo * D:thi * D],
                    idxs_ap=it[:, r, 1, :], num_idxs=M, num_idxs_reg=M,
                    elem_size=nt * D, elem_step=cfg.ELEM, transpose=True,
                )
                nc.vector.tensor_tensor(out=xr[:, tlo:thi, :], in0=g1[:], in1=g2[:], op=OP.add)

            def norm_sub(tlo, thi, xr, ss):
                sq = sqp.tile([128, thi - tlo, M], BF16, tag="sqs", name="sqs")
                xe = xr[:, tlo:thi, :]
                nc.vector.tensor_tensor(out=sq[:], in0=xe, in1=xe, op=OP.mult)
                nc.vector.tensor_reduce(
                    out=ss[:, tlo * cfg.NBG:thi * cfg.NBG],
                    in_=sq[:].rearrange("p t (b n) -> p (t b) n", n=64),
                    op=OP.add, axis=mybir.AxisListType.X)

            def scale_sub(tlo, thi, xr, y):
                sv = bass.AP(y.tensor, y.offset + tlo * cfg.NBG,
                             [y.ap[0], [cfg.NBG, thi - tlo], [1, cfg.NBG], [0, 64]])
                xe = xr[:, tlo:thi, :]
                return nc.vector.tensor_tensor(
                    out=xe.rearrange("p t (b n) -> p t b n", n=64),
                    in0=xe.rearrange("p t (b n) -> p t b n", n=64),
                    in1=sv, op=OP.mult)

            def lstm_step(r, xr, dirn, te, h, c, wq, bq, first=False):
                gd = {}
                for q in (0, 2, 1, 3):  # i, g first: u1 = sig(i)*tanh(g) starts earlier
                    ps = psp.tile([128, M], F32, tag=f"ps{dirn}{q}", name="ps")
                    nc.tensor.matmul(ps[:], lhsT=wih_t[:, r, dirn, q * H:(q + 1) * H],
                                     rhs=xr[:, te, :], start=True, stop=first)
                    if not first:
                        nc.tensor.matmul(ps[:], lhsT=whh_t[:, r, dirn, q * H:(q + 1) * H],
                                         rhs=h[dirn][:], start=False, stop=True)
                    gq = gtp.tile([128, M], GDT, tag=f"g{dirn}{q}", name="gq")
                    nc.scalar.activation(gq[:], ps[:],
                                         AF.Tanh if q == 2 else AF.Sigmoid,
                                         bias=bias_t[:, r, dirn, q:q + 1])
                    gd[q] = gq
                gi, gf, gg, go = gd[0], gd[1], gd[2], gd[3]
                u1 = gtp.tile([128, M], CDT, tag=f"u1{dirn}", name="u1")
                nc.vector.tensor_tensor(out=u1[:], in0=gi[:], in1=gg[:], op=OP.mult)
                if first:
                    c[dirn] = u1  # c(0) = sig(i)*tanh(g); h,c start at zero
                else:
                    u2 = gtp.tile([128, M], CDT, tag=f"u2{dirn}", name="u2")
                    nc.vector.tensor_tensor(out=u2[:], in0=gf[:], in1=c[dirn][:], op=OP.mult)
                    c[dirn] = st.tile([128, M], CDT, tag=f"c{dirn}", name="cn")
                    nc.vector.tensor_tensor(out=c[dirn][:], in0=u1[:], in1=u2[:], op=OP.add)
                th = gtp.tile([128, M], GDT, tag=f"th{dirn}", name="th")
                nc.scalar.activation(th[:], c[dirn][:], AF.Tanh)
                h[dirn] = st.tile([128, M], BF16, tag=f"h{dirn}", name="hn")
                nc.vector.tensor_tensor(out=h[dirn][:], in0=go[:], in1=th[:], op=OP.mult)

            dep_guard = []
            for r in range(R):
                xr = xp.tile([128, T, M], BF16, tag=f"x{r}", name="xr")
                h = {}; c = {}

                if r == 0:
                    # fast-start: per-half norm; fwd steps 0..TH-1 run on e0 only
                    ss = ntp.tile([128, T * cfg.NBG], F32, tag="ss", name="ss")
                    y = ntp.tile([128, T * cfg.NBG], F32, tag="y", name="y")
                    with tc.high_priority(offset=None):
                        gather_half(r, 0, xr)
                    adds_e1 = gather_half(r, 1, xr)
                    with tc.high_priority(offset=None):
                        norm_half(r, 0, xr, ss)
                        newton(ss[:, 0:cfg.TH * cfg.NBG], y[:, 0:cfg.TH * cfg.NBG])
                        sc_e0 = scale_half(r, 0, xr, y)
                    # keep the e0 critical chain free of stealable big DVE ops
                    for a in adds_e1:
                        add_dep_helper(a.ins, sc_e0.ins, sync=False,
                                       reason="startup: e1 add after e0 scale")
                    LEAD = min(6, cfg.TH)  # bwd joins after this many fwd steps
                    for t in range(LEAD):
                        lstm_step(r, xr, 0, t, h, c, wih_t, bias_t, first=(t == 0))
                    norm_half(r, 1, xr, ss)
                    newton(ss[:, cfg.TH * cfg.NBG:], y[:, cfg.TH * cfg.NBG:])
                    sc_e1 = scale_half(r, 1, xr, y)
                    dep_guard.append(sc_e1)
                    for t in range(LEAD, T):
                        lstm_step(r, xr, 0, t, h, c, wih_t, bias_t)
                        lstm_step(r, xr, 1, T - 1 - (t - LEAD), h, c, wih_t, bias_t,
                                  first=(t == LEAD))
                    for t in range(T - LEAD, T):
                        lstm_step(r, xr, 1, T - 1 - t, h, c, wih_t, bias_t)
                else:
                    for e in range(2):
                        for a in gather_half(r, e, xr):
                            if r == 1 and dep_guard:
                                add_dep_helper(a.ins, dep_guard[0].ins, sync=False,
                                               reason="r1 adds after r0 e1 scale")
                    ss = ntp.tile([128, T * cfg.NBG], F32, tag="ss", name="ss")
                    for e in range(2):
                        norm_half(r, e, xr, ss)
                    y = ntp.tile([128, T * cfg.NBG], F32, tag="y", name="y")
                    newton(ss[:], y[:])
                    for e in range(2):
                        scale_half(r, e, xr, y)
                    for t in range(T):
                        for dirn in range(2):
                            te = t if dirn == 0 else T - 1 - t
                            lstm_step(r, xr, dirn, te, h, c, wih_t, bias_t, first=(t == 0))

                for dirn in range(2):
                    rl = gtp.tile([128, M], BF16, tag=f"rl{dirn}", name="rl")
                    nc.vector.tensor_scalar(out=rl[:], in0=h[dirn][:], scalar1=0.0,
                                            scalar2=None, op0=OP.max)
                    sv = ntp.tile([128, cfg.NBG], F32, tag=f"S{dirn}", name="sv")
                    nc.vector.tensor_reduce(
                        out=sv[:], in_=rl[:].rearrange("p (b n) -> p b n", n=64),
                        op=OP.add, axis=mybir.AxisListType.X)
                    nc.sync.dma_start(out=sout[r, dirn], in_=sv[:])

    nc.compile()
    return nc


# ---------------- host side ----------------

def prep_table(cfg: Cfg, embeddings, alignment_list):
    """bt[a] = concat_t embeddings[t, alignment_list[a, t]] with zero rows."""
    T = cfg.T
    al = np.asarray(alignment_list)
    emb = np.asarray(embeddings)
    SPLIT = 32768 if cfg.NROWS_RAW > 32768 else (cfg.NROWS_RAW // 2 + 1)
    body = np.empty((cfg.NROWS_RAW, cfg.ELEM), dtype=ml_dtypes.bfloat16)
    for t in range(T):
        body[:, t * cfg.D:(t + 1) * cfg.D] = emb[t][al[:, t]].astype(ml_dtypes.bfloat16)
    bt = np.zeros((cfg.NROWS, cfg.ELEM), dtype=ml_dtypes.bfloat16)
    bt[1:SPLIT] = body[0:SPLIT - 1]
    bt[SPLIT + 1:cfg.NROWS] = body[SPLIT - 1:cfg.NROWS_RAW]
    return bt, SPLIT


def prep_idx(cfg: Cfg, a_arr, SPLIT):
    """a_arr: [R, M] alignment ids for this core -> [128, R, 2, M//16] int16."""
    R, M = cfg.R, cfg.M
    out = np.zeros((128, R, 2, M // 16), dtype=np.int16)
    for r in range(R):
        a = a_arr[r]
        i1 = np.where(a <= SPLIT - 2, a + 1, 0).astype(np.int16)
        i2 = np.where(a >= SPLIT - 1, a - (SPLIT - 2), 0).astype(np.int16)
        out[:, r, 0, :] = np.tile(i1.reshape(M // 16, 16).T, (8, 1))
        out[:, r, 1, :] = np.tile(i2.reshape(M // 16, 16).T, (8, 1))
    return out


def prep_weights(cfg: Cfg, ins):
    H = cfg.D
    wih = np.zeros((128, cfg.R, 2, 4 * H), dtype=ml_dtypes.bfloat16)
    whh = np.zeros((128, cfg.R, 2, 4 * H), dtype=ml_dtypes.bfloat16)
    bias = np.zeros((128, cfg.R, 2, 4), dtype=np.float32)
    for r in range(cfg.R):
        for dirn, sfx in ((0, "_f"), (1, "_b")):
            wih[:, r, dirn, :] = np.asarray(ins["Wih" + sfx][r]).T.astype(ml_dtypes.bfloat16)
            whh[:, r, dirn, :] = np.asarray(ins["Whh" + sfx][r]).T.astype(ml_dtypes.bfloat16)
            b = (np.asarray(ins["bih" + sfx][r]) + np.asarray(ins["bhh" + sfx][r])).astype(np.float32)
            bias[:, r, dirn, :] = b.reshape(4, H).T
    return wih, whh, bias


def finalize(cfg: Cfg, s_cores, ins, nb_total):
    """s_cores: list of [R, 2, 128, NBG] per core -> output [B, OUT] f32."""
    fc_W = np.asarray(ins["fc_W"], np.float64)
    fc_b = np.asarray(ins["fc_b"], np.float64)
    Wsum = np.asarray(ins["W1"], np.float64) + np.asarray(ins["W2"], np.float64)
    Wrel = np.asarray(ins["Wrel"], np.float64)
    outs = []
    for s in s_cores:
        tot = None
        for r in range(cfg.R):
            s_cat = np.concatenate([s[r, 1], s[r, 0]], axis=0).astype(np.float64)  # [2H, NBG]
            o = fc_W[r] @ s_cat + nb_total * fc_b[r][:, None]                      # [OUT, NBG]
            inf = Wrel[r].T @ (Wsum[r].T @ o)                                      # [INF, NBG]
            tot = inf if tot is None else tot + inf
        outs.append(tot.T)  # [NBG, INF] -> batch-local rows
    return np.concatenate(outs, axis=0).astype(np.float32)


# ---------------- self-contained entry point ----------------

_CACHE = {}


def kernel(**inputs):
    """Full-inputs -> full-output Trainium kernel for the Dynamic Influence
    Model. Shards the batch (B=64) over 8 NeuronCores; each core gathers its
    own neighbor sequences from a replicated sequence-major embedding table,
    runs the per-relation BiLSTMs on-device, and returns sum_nb relu(h);
    the tiny trailing FC chain is applied on the host in float64 (exactly
    equivalent algebra - the neighbor sum commutes with the linear layers).
    """
    from concourse.bass_utils import run_bass_kernel_spmd

    cfg = _CACHE.get("cfg")
    if cfg is None:
        cfg = Cfg()
        _CACHE["cfg"] = cfg
    nc = _CACHE.get("nc")
    if nc is None:
        nc = build_nc(cfg)
        _CACHE["nc"] = nc

    bt, SPLIT = prep_table(cfg, inputs["embeddings"], inputs["alignment_list"])
    wih, whh, bias = prep_weights(cfg, inputs)
    neighbors = np.asarray(inputs["neighbors"])
    in_maps = []
    for core in range(8):
        a_arr = neighbors[core * 8:(core + 1) * 8].transpose(1, 0, 2).reshape(cfg.R, cfg.M)
        idx = prep_idx(cfg, a_arr, SPLIT)
        in_maps.append({"table": bt, "idxs": idx, "wih": wih, "whh": whh, "bias": bias})

    res = run_bass_kernel_spmd(nc, in_maps, list(range(8)))
    s_cores = [res.results[i]["sout"] for i in range(8)]
    return finalize(cfg, s_cores, inputs, nb_total=64)

